# revision 25
# baseline (speedup 1.0000x reference)
"""Multi-head causal attention (B=4, S=2048, D=1024, H=16) on 8 Trainium2 cores.

Sharding: core c -> (batch b = c//2, head-half hh = c%2), i.e. each core computes
attention for one batch and 8 of the 16 heads, plus the partial output
projection against its row-shard of Wo. Host sums the per-batch core pair
(the Wo row-shard all-reduce) and transposes.

On-device layout (per core, all matmul operands bf16, accumulation fp32):
  - q/k projections produce qT/kT [head-pair 128, S] (features on partitions)
  - scores are computed transposed: S^T[t, s] tiles (keys on partitions) so
    exp() on ScalarE writes P^T directly, and softmax denominators come for
    free from a ones-column appended to V during the P^T @ V_aug matmul.
  - causal masking: tiles strictly above the diagonal are never computed;
    diagonal tiles get a 0/1 mask multiply post-exp.
  - units are ordered u-major (all pairs' left s-half before any right half)
    so the left-half output projection unlocks mid-schedule as PE filler.
  - projections are emitted through a need-ordered deque consumed one block
    per score window, so the PE always has dependency-free work while
    ScalarE drains exps (keeps the PE p-state at full clock).
"""

import os
import sys
from collections import deque
from contextlib import ExitStack

for _p in (
    "/opt/trn_rl_repo/concourse",
    "/root/.axon_site/_ro/trn_rl_repo/concourse",
):
    if os.path.isdir(_p) and _p not in sys.path:
        sys.path.append(_p)

import numpy as np
import ml_dtypes

BF16 = ml_dtypes.bfloat16

HD = 64          # head dim
NH = 8           # heads per core
G = NH // 2      # head-pair groups (2 heads -> 128 partitions)
EC = NH * HD // 128  # o^T feature chunks (=4)


def build_nc(S, D):
    import concourse.tile as tile
    from concourse import bacc, mybir

    f32 = mybir.dt.float32
    bf16 = mybir.dt.bfloat16
    Exp = mybir.ActivationFunctionType.Exp
    add = mybir.AluOpType.add
    mult = mybir.AluOpType.mult

    KC = D // 128    # contraction chunks over model dim
    ST = S // 128    # 128-token tiles
    SC = S // 512    # 512-token projection s-chunks
    NU = max(1, SC // 2)          # s-range units per head pair
    UW = (SC // NU) * 512         # unit width in columns

    nc = bacc.Bacc(None, target_bir_lowering=False)

    xq = nc.dram_tensor("xq", [D, S], bf16, kind="ExternalInput")
    xk = nc.dram_tensor("xk", [D, S], bf16, kind="ExternalInput")
    xv = nc.dram_tensor("xv", [D, S], bf16, kind="ExternalInput")
    wq = nc.dram_tensor("wq", [D, NH * HD], bf16, kind="ExternalInput")
    wk = nc.dram_tensor("wk", [D, NH * HD], bf16, kind="ExternalInput")
    wv = nc.dram_tensor("wv", [D, NH * HD], bf16, kind="ExternalInput")
    wo = nc.dram_tensor("wo", [NH * HD, D], bf16, kind="ExternalInput")
    bqd = nc.dram_tensor("bq", [128, G], f32, kind="ExternalInput")
    bkd = nc.dram_tensor("bk", [128, G], f32, kind="ExternalInput")
    bvd = nc.dram_tensor("bv", [128, NH, HD], f32, kind="ExternalInput")
    bod = nc.dram_tensor("bo", [128, D // 128], f32, kind="ExternalInput")
    maskd = nc.dram_tensor("mask", [128, 128], bf16, kind="ExternalInput")
    out = nc.dram_tensor("out", [D, S], f32, kind="ExternalOutput")

    with tile.TileContext(nc) as tc, ExitStack() as ctx:
        const_pool = ctx.enter_context(tc.tile_pool(name="const", bufs=1))
        wqk_pool = ctx.enter_context(tc.tile_pool(name="wqk", bufs=4))
        xpool = ctx.enter_context(tc.tile_pool(name="x", bufs=2))
        qk_pool = ctx.enter_context(tc.tile_pool(name="qk", bufs=1))
        v_pool = ctx.enter_context(tc.tile_pool(name="v", bufs=1))
        pt_pool = ctx.enter_context(tc.tile_pool(name="pt", bufs=1))
        o_pool = ctx.enter_context(tc.tile_pool(name="o", bufs=1))
        r_pool = ctx.enter_context(tc.tile_pool(name="r", bufs=3))
        ou_pool = ctx.enter_context(tc.tile_pool(name="ou", bufs=3))
        out_pool = ctx.enter_context(tc.tile_pool(name="outp", bufs=3))
        ps_mm = ctx.enter_context(tc.tile_pool(name="psmm", bufs=2, space="PSUM"))
        ps_st = ctx.enter_context(tc.tile_pool(name="psst", bufs=3, space="PSUM"))

        # ---- critical-path DMAs first, ordered so each projection chain's
        # operands land just before the chain runs: wq+xq0 for the first
        # chain, wk+xk0 transferring concurrently for the second.
        wqg0 = wqk_pool.tile([128, KC, 256], bf16, tag="wqk", name="wq_0")
        wkg0 = wqk_pool.tile([128, KC, 256], bf16, tag="wqk", name="wk_0")
        nc.sync.dma_start(
            wqg0[:], wq[:, 0:256].rearrange("(kc p) m -> p kc m", p=128)
        )
        bq_sb = const_pool.tile([128, G], f32)
        bk_sb = const_pool.tile([128, G], f32)

        wv_sb = const_pool.tile([128, KC, NH * HD], bf16)
        wo_sb = const_pool.tile([128, EC, D], bf16)
        bv_sb = const_pool.tile([128, NH, HD], f32)
        bo_sb = const_pool.tile([128, D // 128], f32)
        mask_sb = const_pool.tile([128, 128], bf16)

        qT = qk_pool.tile([128, G, S], bf16, tag="qT")
        kT = qk_pool.tile([128, G, S], bf16, tag="kT")
        v_sb = v_pool.tile([128, ST, NH, HD + 1], bf16, tag="v")
        oT = o_pool.tile([128, EC, S], bf16, tag="oT")

        # ---- projection emitters ----
        def load_xslice(xsrc, sc, name):
            """One batched DMA: all KC chunks of a 512-token column slice."""
            xt = xpool.tile([128, KC, 512], bf16, tag="xb", name=name)
            nc.sync.dma_start(
                xt[:],
                xsrc.rearrange("(kc p) s -> p kc s", p=128)[
                    :, :, sc * 512 : (sc + 1) * 512
                ],
            )
            return xt

        def kchain(psum_ap, lhs_of, rhs_of, n):
            for c in range(n):
                nc.tensor.matmul(
                    psum_ap, lhs_of(c), rhs_of(c),
                    start=(c == 0), stop=(c == n - 1),
                )

        def proj_qk_dma(pname, xsrc, gh, sc):
            return load_xslice(xsrc, sc, f"x{pname}{gh}_{sc}")

        def proj_qk_mms(xt, wsb, bsb, dst, gh, sc, pname):
            for i in range(2):
                psum = ps_mm.tile([128, 512], f32, tag="mm", name=f"p{pname}{gh}_{sc}_{i}")
                kchain(
                    psum[:],
                    lambda c, i=i: wsb[:, c, i * 128 : (i + 1) * 128],
                    lambda c: xt[:, c, :],
                    KC,
                )
                g = 2 * gh + i
                nc.vector.tensor_scalar_add(
                    dst[:, g, sc * 512 : (sc + 1) * 512], psum[:], bsb[:, g : g + 1]
                )

        def proj_v_mms(xt, sc, half):
            """Half an s-chunk of the V projection: 2 psum chains (256 tokens).
            The two halves of an s-chunk share one x-slice DMA."""
            for i in range(2):
                psum = ps_mm.tile([128, NH, HD], f32, tag="mm", name=f"psv{sc}_{half}_{i}")
                kchain(
                    psum[:],
                    lambda c, i=i, half=half: xt[
                        :, c, half * 256 + i * 128 : half * 256 + (i + 1) * 128
                    ],
                    lambda c: wv_sb[:, c, :],
                    KC,
                )
                sti = sc * 4 + half * 2 + i
                nc.vector.tensor_tensor(
                    v_sb[:, sti, :, 0:HD], psum[:], bv_sb[:], add
                )

        def make_gh1_weights():
            wqg = wqk_pool.tile([128, KC, 256], bf16, tag="wqk", name="wq_1")
            wkg = wqk_pool.tile([128, KC, 256], bf16, tag="wqk", name="wk_1")
            nc.sync.dma_start(
                wqg[:], wq[:, 256:512].rearrange("(kc p) m -> p kc m", p=128)
            )
            nc.sync.dma_start(
                wkg[:], wk[:, 256:512].rearrange("(kc p) m -> p kc m", p=128)
            )
            return wqg, wkg

        # ---- unit geometry ----
        def unit_geom(u):
            s_lo, s_hi = u * UW, min((u + 1) * UW, S)
            ts, offs, cols = [], {}, 0
            for t in range(0, s_hi // 128):
                w = s_hi - max(t * 128, s_lo)
                if w <= 0:
                    continue
                ts.append(t)
                offs[t] = cols
                cols += w
            return s_lo, s_hi, ts, offs, cols

        def st_unit(g, u, filler, cadence=1):
            """S^T + exp + mask for both heads of pair g over unit u's columns.
            One filler block is emitted after each exp window to keep the PE
            busy while ScalarE drains; diagonal-tile masks are emitted in the
            window where their columns complete (spreads DVE load)."""
            s_lo, s_hi, ts, offs, cols = unit_geom(u)
            pts = []
            for j in range(2):
                pt = pt_pool.tile(
                    [128, cols], bf16, tag=f"ph{u}_{j}", name=f"pt{g}_{u}_{j}", bufs=1
                )
                pts.append(pt)
            chunks = []  # (t, s_from, win, win_off, wlen)
            pos = 0
            for t in ts:
                s_from = max(t * 128, s_lo)
                rem = s_hi - s_from
                while rem:
                    wlen = min(512 - pos % 512, rem)
                    chunks.append((t, s_from, pos // 1024, pos % 1024, wlen))
                    pos += wlen
                    s_from += wlen
                    rem -= wlen
            nwin = (pos + 1023) // 1024
            covered = 0
            masks_todo = [t for t in ts if t * 128 >= s_lo]
            for w in range(nwin):
                wchunks = [c for c in chunks if c[2] == w]
                wcols = sum(c[4] for c in wchunks)
                gbase = 1024 * w
                stt = []
                for j in range(2):
                    st_t = ps_st.tile(
                        [128, 1024], f32, tag="st", name=f"st{g}_{u}_{w}_{j}"
                    )
                    stt.append(st_t)
                for t, s_from, _, woff, wlen in wchunks:
                    for j in range(2):
                        ro = j * HD
                        nc.tensor.matmul(
                            stt[j][:, woff : woff + wlen],
                            kT[ro : ro + HD, g, t * 128 : t * 128 + 128],
                            qT[ro : ro + HD, g, s_from : s_from + wlen],
                            start=True,
                            stop=True,
                        )
                for j in range(2):
                    nc.scalar.activation(
                        pts[j][:, gbase : gbase + wcols],
                        stt[j][:, 0:wcols],
                        Exp,
                        scale=1.0 / np.sqrt(HD),
                    )
                covered += wcols
                while masks_todo and offs[masks_todo[0]] + 128 <= covered:
                    t = masks_todo.pop(0)
                    for j in range(2):
                        nc.vector.tensor_tensor(
                            pts[j][:, offs[t] : offs[t] + 128],
                            pts[j][:, offs[t] : offs[t] + 128],
                            mask_sb[:],
                            mult,
                        )
                if filler is not None and (cadence == 1 or w % cadence == 1):
                    blk = filler()
                    if blk is not None:
                        blk()
            assert not masks_todo
            return pts

        def outproj_sc(dc, sc):
            """Output projection for one (128-feature, 512-token) tile + DMA."""
            ot = out_pool.tile([128, 512], f32, tag="ot", name=f"ot{dc}_{sc}")
            psum = ps_mm.tile([128, 512], f32, tag="mm", name=f"po{dc}_{sc}")
            kchain(
                psum[:],
                lambda c: wo_sb[:, c, dc * 128 : (dc + 1) * 128],
                lambda c: oT[:, c, sc * 512 : (sc + 1) * 512],
                EC,
            )
            nc.vector.tensor_scalar_add(ot[:], psum[:], bo_sb[:, dc : dc + 1])
            nc.sync.dma_start(
                out[dc * 128 : (dc + 1) * 128, sc * 512 : (sc + 1) * 512], ot[:]
            )

        def av_group(g, u, pts, j, g4, norm_q):
            """O^T accumulation for head (g, j), s-group g4. The normalize
            multiply is deferred via norm_q so the in-order DVE never blocks
            on the GpSimd broadcast while later PSUM evacuations queue."""
            s_lo, s_hi, ts, offs, cols = unit_geom(u)
            h = 2 * g + j
            ro = j * HD
            av = ps_mm.tile([128, 512], f32, tag="mm", name=f"av{g}_{u}_{j}_{g4}")
            tlist = [t for t in ts if t * 128 < (g4 + 1) * 512]
            for ci, t in enumerate(tlist):
                lo = max(g4 * 512, t * 128)
                n = (g4 + 1) * 512 - lo
                col = offs[t] + lo - max(t * 128, s_lo)
                nc.tensor.matmul(
                    av[0 : HD + 1, lo - g4 * 512 : lo - g4 * 512 + n],
                    v_sb[:, t, h, :],
                    pts[j][:, col : col + n],
                    start=(ci == 0),
                    stop=(ci == len(tlist) - 1),
                )
            # evacuate PSUM quickly (frees the accumulator slot). The denom
            # reciprocal runs on a [128,4] transpose (a [1,512] DVE op is
            # ~20x slower on hardware), then is broadcast across partitions.
            osb = ou_pool.tile([HD + 1, 512], f32, tag="ou", name=f"ou_{g}_{u}_{j}_{g4}")
            nc.vector.tensor_copy(osb[:], av[0 : HD + 1, :])
            rs = r_pool.tile([128, 4], f32, tag="rs", name=f"rs_{g}_{u}_{j}_{g4}")
            nc.gpsimd.dma_start(rs[:], osb[HD : HD + 1, :])
            rr = r_pool.tile([128, 4], f32, tag="rrt", name=f"rrt_{g}_{u}_{j}_{g4}")
            nc.vector.reciprocal(rr[:], rs[:])
            rrow = r_pool.tile([1, 512], f32, tag="rr", name=f"rr_{g}_{u}_{j}_{g4}")
            nc.gpsimd.dma_start(rrow[:], rr[:])
            r64 = r_pool.tile(
                [HD, 512], f32, tag="r64", name=f"r64_{g}_{u}_{j}_{g4}"
            )
            nc.gpsimd.partition_broadcast(r64[:], rrow[:])

            def norm():
                nc.vector.tensor_tensor(
                    oT[ro : ro + HD, g, g4 * 512 : (g4 + 1) * 512],
                    osb[0:HD, :],
                    r64[:],
                    mult,
                )
            norm_q.append(norm)

        def av_unit(g, u, pts, g4_desc=False):
            s_lo, s_hi = u * UW, min((u + 1) * UW, S)
            norm_q = []
            g4s = range(s_lo // 512, s_hi // 512)
            for g4 in (reversed(g4s) if g4_desc else g4s):
                for j in range(2):
                    av_group(g, u, pts, j, g4, norm_q)
                    while len(norm_q) > 1:
                        norm_q.pop(0)()
            while norm_q:
                norm_q.pop(0)()

        # ---- emission ----
        # Upfront: pairs 0-1 q/k for the left s-half only, with x DMAs
        # interleaved ahead of the chains that consume them. The exp table
        # preloads on ScalarE during the initial DMA wait.
        xts = {}
        xts["q0"] = proj_qk_dma("q", xq, 0, 0)
        nc.sync.dma_start(
            wkg0[:], wk[:, 0:256].rearrange("(kc p) m -> p kc m", p=128)
        )
        xts["k0"] = proj_qk_dma("k", xk, 0, 0)
        nc.sync.dma_start(bq_sb[:], bqd[:])
        nc.sync.dma_start(bk_sb[:], bkd[:])
        warm = r_pool.tile([128, 4], f32, tag="rs", name="warm")
        nc.vector.memset(warm[:], 1.0)
        nc.scalar.activation(warm[:], warm[:], Exp, scale=1.0)
        proj_qk_mms(xts["q0"], wqg0, bq_sb, qT, 0, 0, "q")
        xts["q1"] = proj_qk_dma("q", xq, 0, 1)
        proj_qk_mms(xts["k0"], wkg0, bk_sb, kT, 0, 0, "k")
        xts["k1"] = proj_qk_dma("k", xk, 0, 1)
        proj_qk_mms(xts["q1"], wqg0, bq_sb, qT, 0, 1, "q")
        proj_qk_mms(xts["k1"], wkg0, bk_sb, kT, 0, 1, "k")

        # Remaining const DMAs (needed from the first windows / v blocks on).
        nc.sync.dma_start(mask_sb[:], maskd[:])
        nc.sync.dma_start(wv_sb[:], wv.rearrange("(kc p) m -> p kc m", p=128))
        nc.sync.dma_start(bv_sb[:], bvd[:])
        nc.sync.dma_start(wo_sb[:], wo.rearrange("(ec p) d -> p ec d", p=128))
        nc.sync.dma_start(bo_sb[:], bod[:])
        nc.vector.memset(v_sb[:, :, :, HD : HD + 1], 1.0)
        wqg1, wkg1 = make_gh1_weights()

        # Filler blocks: (label, dma_fn|None, mm_fn). The x-slice DMA of the
        # next dma-bearing block is issued one block ahead of its compute so
        # the transfer hides under the previous block's matmuls.
        emitted = {"q0s0", "q0s1", "k0s0", "k0s1"}
        blocks = []
        _slices = {}

        def add_qk(gh, nm, sc):
            lbl = f"{nm}{gh}s{sc}"
            xsrc = xq if nm == "q" else xk
            wsb = (wqg0 if nm == "q" else wkg0) if gh == 0 else (
                wqg1 if nm == "q" else wkg1)
            bsb = bq_sb if nm == "q" else bk_sb
            dst = qT if nm == "q" else kT

            def dma(lbl=lbl):
                _slices[lbl] = proj_qk_dma(nm, xsrc, gh, sc)

            def mms(lbl=lbl):
                proj_qk_mms(_slices.pop(lbl), wsb, bsb, dst, gh, sc, nm)
            blocks.append((lbl, dma, mms))

        def add_v(sc, half):
            lbl = f"v{sc}"

            def dma(sc=sc):
                _slices[f"xv{sc}"] = load_xslice(xv, sc, f"xv_{sc}")

            def mms(sc=sc, half=half):
                xt = _slices[f"xv{sc}"] if half == 0 else _slices.pop(f"xv{sc}")
                proj_v_mms(xt, sc, half)
            blocks.append((lbl, dma if half == 0 else None, mms))

        # first-need order for g-major units with st-ahead: st(0,1) is
        # emitted during iteration 0, so gh0 s23 comes first.
        for nm in ("q", "k"):
            for sc in (2, 3):
                add_qk(0, nm, sc)
        for sc in (0, 1, 2, 3):
            for half in (0, 1):
                add_v(sc, half)
        for sc in (0, 1, 2, 3):
            for nm in ("q", "k"):
                add_qk(1, nm, sc)

        state = {"dma": 0, "mm": 0}

        def _advance_dma():
            # issue x DMAs up to one dma-bearing block past the next compute
            want = state["mm"] + 2
            while state["dma"] < min(want, len(blocks)):
                i = state["dma"]
                if blocks[i][1] is not None:
                    blocks[i][1]()
                state["dma"] += 1

        def run_next_block():
            i = state["mm"]
            lbl, _, mms = blocks[i]
            state["mm"] += 1
            _advance_dma()
            mms()
            emitted.add(lbl)

        _advance_dma()

        def take_filler():
            if state["mm"] < len(blocks):
                return run_next_block
            if fill_q:
                return fill_q.popleft()[1]
            return None

        fill_q = deque()

        def push(label, fn):
            def run(label=label, fn=fn):
                fn()
                emitted.add(label)
            fill_q.append((label, run))

        def prep(labels):
            while state["mm"] < len(blocks) and not all(
                l in emitted for l in labels
            ):
                run_next_block()
            while fill_q and not all(l in emitted for l in labels):
                fill_q.popleft()[1]()

        def st_needs(g, u):
            gh = g // 2
            scs = (0, 1) if u == 0 else (0, 1, 2, 3)
            qscs = (0, 1) if u == 0 else (2, 3)
            return [f"q{gh}s{sc}" for sc in qscs] + [f"k{gh}s{sc}" for sc in scs]

        def av_needs(u):
            scs = (0, 1) if u == 0 else (2, 3)
            return [f"v{sc}" for sc in scs]

        units = [(g, u) for g in range(G) for u in range(NU)]

        reserve = []

        def push_outproj_left():
            for sc in range(UW // 512):
                for dc in range(D // 128):
                    if sc == 1 and dc >= 4:
                        # held back: emitted between the last unit's windows
                        # and its AV groups, covering the ScalarE exp backlog
                        reserve.append(lambda dc=dc, sc=sc: outproj_sc(dc, sc))
                    else:
                        push(f"op{dc}_{sc}", lambda dc=dc, sc=sc: outproj_sc(dc, sc))

        # Software pipeline: unit i+1's score windows are emitted before unit
        # i's AV groups so ScalarE exps overlap the AV matmuls. At (3,0) the
        # order flips so the left-half output projection unlocks before the
        # final unit's windows need fillers.
        pts_next = st_unit(*units[0], take_filler, cadence=2)
        for i, (g, u) in enumerate(units):
            pts_cur = pts_next
            last = i == len(units) - 1
            ahead = not last and units[i + 1]
            if ahead and (g, u) != (G - 1, 0):
                prep(st_needs(*ahead))
                pts_next = st_unit(*ahead, take_filler,
                                   cadence=2 if i + 1 < 4 else 1)
            prep(av_needs(u))
            if last:
                # reserved fillers cover the ScalarE exp backlog of the last
                # windows; AV groups run g4-descending so outproj(sc3) hides
                # under the g4=2 AV+normalize chain and outproj(sc2) under
                # sc3's — the tail never waits a full r-dance latency.
                while fill_q:
                    fill_q.popleft()[1]()
                for fn in reserve:
                    fn()
                reserve.clear()
                av_unit(g, u, pts_cur, g4_desc=True)
                for dc in range(D // 128):
                    outproj_sc(dc, 3)
                for dc in range(D // 128):
                    outproj_sc(dc, 2)
            else:
                av_unit(g, u, pts_cur)
            if (g, u) == (G - 1, 0):
                push_outproj_left()
                prep(st_needs(*ahead))
                pts_next = st_unit(*ahead, take_filler)
        while fill_q:
            fill_q.popleft()[1]()

    nc.compile()
    return nc


def core_inputs(queries, keys, values, Wq, bq, Wk, bk, Wv, bv, Wo, bo, b, hh):
    """Build the per-core input map (host-side sharding + bf16 cast)."""
    D = queries.shape[2]
    hs = slice(hh * NH, hh * NH + NH)

    def xt(x):
        return np.ascontiguousarray(x[b].astype(BF16).T)

    def wcat(W):
        return np.ascontiguousarray(
            np.transpose(W[hs], (1, 0, 2)).reshape(D, NH * HD).astype(BF16)
        )

    def bstack(bias):
        return np.ascontiguousarray(
            bias[hs].reshape(G, 128).T.astype(np.float32)
        )

    mask = np.triu(np.ones((128, 128), np.float32)).astype(BF16)
    return {
        "xq": xt(queries),
        "xk": xt(keys),
        "xv": xt(values),
        "wq": wcat(Wq),
        "wk": wcat(Wk),
        "wv": wcat(Wv),
        "wo": np.ascontiguousarray(Wo[hh * NH * HD : (hh + 1) * NH * HD].astype(BF16)),
        "bq": bstack(bq),
        "bk": bstack(bk),
        "bv": np.ascontiguousarray(
            np.broadcast_to(bv[hs].reshape(1, NH, HD), (128, NH, HD)).astype(np.float32)
        ),
        "bo": np.ascontiguousarray(
            (bo.reshape(D // 128, 128) / 2.0).T.astype(np.float32)
        ),
        "mask": mask,
    }


_NC_CACHE = {}


def _get_nc(S, D):
    key = (S, D)
    if key not in _NC_CACHE:
        _NC_CACHE[key] = build_nc(S, D)
    return _NC_CACHE[key]


def kernel(keys, queries, values, Wq, bq, Wk, bk, Wv, bv, Wo, bo, _trace=False):
    keys, queries, values = (np.asarray(a) for a in (keys, queries, values))
    Wq, bq, Wk, bk, Wv, bv, Wo, bo = (
        np.asarray(a) for a in (Wq, bq, Wk, bk, Wv, bv, Wo, bo)
    )
    B, S, D = queries.shape
    nc = _get_nc(S, D)

    in_maps = [
        core_inputs(queries, keys, values, Wq, bq, Wk, bk, Wv, bv, Wo, bo, c // 2, c % 2)
        for c in range(8)
    ]
    from concourse.bass_utils import run_bass_kernel_spmd

    res = run_bass_kernel_spmd(
        nc, in_maps, core_ids=list(range(8)), trace=_trace
    )
    kernel.last_result = res
    outs = [r["out"] for r in res.results]
    out = np.empty((B, S, D), np.float32)
    for b in range(B):
        out[b] = (outs[2 * b] + outs[2 * b + 1]).T
    return out


# revision 30
# speedup vs baseline: 1.0121x; 1.0121x over previous
"""Multi-head causal attention (B=4, S=2048, D=1024, H=16) on 8 Trainium2 cores.

Sharding: core c -> (batch b = c//2, head-half hh = c%2), i.e. each core computes
attention for one batch and 8 of the 16 heads, plus the partial output
projection against its row-shard of Wo. Host sums the per-batch core pair
(the Wo row-shard all-reduce) and transposes.

On-device layout (per core, all matmul operands bf16, accumulation fp32):
  - q/k projections produce qT/kT [head-pair 128, S] (features on partitions)
  - scores are computed transposed: S^T[t, s] tiles (keys on partitions) so
    exp() on ScalarE writes P^T directly, and softmax denominators come for
    free from a ones-column appended to V during the P^T @ V_aug matmul.
  - causal masking: tiles strictly above the diagonal are never computed;
    diagonal tiles get a 0/1 mask multiply post-exp.
  - units are ordered u-major (all pairs' left s-half before any right half)
    so the left-half output projection unlocks mid-schedule as PE filler.
  - projections are emitted through a need-ordered deque consumed one block
    per score window, so the PE always has dependency-free work while
    ScalarE drains exps (keeps the PE p-state at full clock).
"""

import os
import sys
from collections import deque
from contextlib import ExitStack

for _p in (
    "/opt/trn_rl_repo/concourse",
    "/root/.axon_site/_ro/trn_rl_repo/concourse",
):
    if os.path.isdir(_p) and _p not in sys.path:
        sys.path.append(_p)

import numpy as np
import ml_dtypes

BF16 = ml_dtypes.bfloat16

HD = 64          # head dim
NH = 8           # heads per core
G = NH // 2      # head-pair groups (2 heads -> 128 partitions)
EC = NH * HD // 128  # o^T feature chunks (=4)


def build_nc(S, D):
    import concourse.tile as tile
    from concourse import bacc, mybir

    f32 = mybir.dt.float32
    bf16 = mybir.dt.bfloat16
    Exp = mybir.ActivationFunctionType.Exp
    add = mybir.AluOpType.add
    mult = mybir.AluOpType.mult

    KC = D // 128    # contraction chunks over model dim
    ST = S // 128    # 128-token tiles
    SC = S // 512    # 512-token projection s-chunks
    NU = max(1, SC // 2)          # s-range units per head pair
    UW = (SC // NU) * 512         # unit width in columns

    nc = bacc.Bacc(None, target_bir_lowering=False)

    xq = nc.dram_tensor("xq", [D, S], bf16, kind="ExternalInput")
    xk = nc.dram_tensor("xk", [D, S], bf16, kind="ExternalInput")
    xv = nc.dram_tensor("xv", [D, S], bf16, kind="ExternalInput")
    wq = nc.dram_tensor("wq", [D, NH * HD], bf16, kind="ExternalInput")
    wk = nc.dram_tensor("wk", [D, NH * HD], bf16, kind="ExternalInput")
    wv = nc.dram_tensor("wv", [D, NH * HD], bf16, kind="ExternalInput")
    wo = nc.dram_tensor("wo", [NH * HD, D], bf16, kind="ExternalInput")
    bqd = nc.dram_tensor("bq", [128, G], f32, kind="ExternalInput")
    bkd = nc.dram_tensor("bk", [128, G], f32, kind="ExternalInput")
    bvd = nc.dram_tensor("bv", [128, NH, HD], f32, kind="ExternalInput")
    bod = nc.dram_tensor("bo", [128, D // 128], f32, kind="ExternalInput")
    maskd = nc.dram_tensor("mask", [128, 128], bf16, kind="ExternalInput")
    out = nc.dram_tensor("out", [D, S], f32, kind="ExternalOutput")

    with tile.TileContext(nc) as tc, ExitStack() as ctx:
        const_pool = ctx.enter_context(tc.tile_pool(name="const", bufs=1))
        wqk_pool = ctx.enter_context(tc.tile_pool(name="wqk", bufs=4))
        xpool = ctx.enter_context(tc.tile_pool(name="x", bufs=2))
        qk_pool = ctx.enter_context(tc.tile_pool(name="qk", bufs=1))
        v_pool = ctx.enter_context(tc.tile_pool(name="v", bufs=1))
        pt_pool = ctx.enter_context(tc.tile_pool(name="pt", bufs=1))
        o_pool = ctx.enter_context(tc.tile_pool(name="o", bufs=1))
        r_pool = ctx.enter_context(tc.tile_pool(name="r", bufs=3))
        ou_pool = ctx.enter_context(tc.tile_pool(name="ou", bufs=3))
        out_pool = ctx.enter_context(tc.tile_pool(name="outp", bufs=3))
        ps_mm = ctx.enter_context(tc.tile_pool(name="psmm", bufs=2, space="PSUM"))
        ps_st = ctx.enter_context(tc.tile_pool(name="psst", bufs=3, space="PSUM"))

        # ---- critical-path DMAs first, ordered so each projection chain's
        # operands land just before the chain runs: wq+xq0 for the first
        # chain, wk+xk0 transferring concurrently for the second.
        wqg0 = wqk_pool.tile([128, KC, 256], bf16, tag="wqk", name="wq_0")
        wkg0 = wqk_pool.tile([128, KC, 256], bf16, tag="wqk", name="wk_0")
        nc.sync.dma_start(
            wqg0[:], wq[:, 0:256].rearrange("(kc p) m -> p kc m", p=128)
        )
        bq_sb = const_pool.tile([128, G], f32)
        bk_sb = const_pool.tile([128, G], f32)

        wv_sb = const_pool.tile([128, KC, NH * HD], bf16)
        wo_sb = const_pool.tile([128, EC, D], bf16)
        bv_sb = const_pool.tile([128, NH, HD], f32)
        bo_sb = const_pool.tile([128, D // 128], f32)
        mask_sb = const_pool.tile([128, 128], bf16)

        qT = qk_pool.tile([128, G, S], bf16, tag="qT")
        kT = qk_pool.tile([128, G, S], bf16, tag="kT")
        v_sb = v_pool.tile([128, ST, NH, HD + 1], bf16, tag="v")
        oT = o_pool.tile([128, EC, S], bf16, tag="oT")

        # ---- projection emitters ----
        def load_xslice(xsrc, sc, name):
            """One batched DMA: all KC chunks of a 512-token column slice."""
            xt = xpool.tile([128, KC, 512], bf16, tag="xb", name=name)
            nc.sync.dma_start(
                xt[:],
                xsrc.rearrange("(kc p) s -> p kc s", p=128)[
                    :, :, sc * 512 : (sc + 1) * 512
                ],
            )
            return xt

        def kchain(psum_ap, lhs_of, rhs_of, n):
            for c in range(n):
                nc.tensor.matmul(
                    psum_ap, lhs_of(c), rhs_of(c),
                    start=(c == 0), stop=(c == n - 1),
                )

        def proj_qk_dma(pname, xsrc, gh, sc):
            return load_xslice(xsrc, sc, f"x{pname}{gh}_{sc}")

        def proj_qk_mms(xt, wsb, bsb, dst, gh, sc, pname):
            for i in range(2):
                psum = ps_mm.tile([128, 512], f32, tag="mm", name=f"p{pname}{gh}_{sc}_{i}")
                kchain(
                    psum[:],
                    lambda c, i=i: wsb[:, c, i * 128 : (i + 1) * 128],
                    lambda c: xt[:, c, :],
                    KC,
                )
                g = 2 * gh + i
                nc.vector.tensor_scalar_add(
                    dst[:, g, sc * 512 : (sc + 1) * 512], psum[:], bsb[:, g : g + 1]
                )

        def proj_v_mms(xt, sc, half):
            """Half an s-chunk of the V projection: 2 psum chains (256 tokens).
            The two halves of an s-chunk share one x-slice DMA."""
            for i in range(2):
                psum = ps_mm.tile([128, NH, HD], f32, tag="mm", name=f"psv{sc}_{half}_{i}")
                kchain(
                    psum[:],
                    lambda c, i=i, half=half: xt[
                        :, c, half * 256 + i * 128 : half * 256 + (i + 1) * 128
                    ],
                    lambda c: wv_sb[:, c, :],
                    KC,
                )
                sti = sc * 4 + half * 2 + i
                nc.vector.tensor_tensor(
                    v_sb[:, sti, :, 0:HD], psum[:], bv_sb[:], add
                )

        def make_gh1_weights():
            wqg = wqk_pool.tile([128, KC, 256], bf16, tag="wqk", name="wq_1")
            wkg = wqk_pool.tile([128, KC, 256], bf16, tag="wqk", name="wk_1")
            nc.sync.dma_start(
                wqg[:], wq[:, 256:512].rearrange("(kc p) m -> p kc m", p=128)
            )
            nc.sync.dma_start(
                wkg[:], wk[:, 256:512].rearrange("(kc p) m -> p kc m", p=128)
            )
            return wqg, wkg

        # ---- unit geometry ----
        def unit_geom(u):
            s_lo, s_hi = u * UW, min((u + 1) * UW, S)
            ts, offs, cols = [], {}, 0
            for t in range(0, s_hi // 128):
                w = s_hi - max(t * 128, s_lo)
                if w <= 0:
                    continue
                ts.append(t)
                offs[t] = cols
                cols += w
            return s_lo, s_hi, ts, offs, cols

        def st_unit(g, u, filler, cadence=1):
            """S^T + exp + mask for both heads of pair g over unit u's columns.
            One filler block is emitted after each exp window to keep the PE
            busy while ScalarE drains; diagonal-tile masks are emitted in the
            window where their columns complete (spreads DVE load)."""
            s_lo, s_hi, ts, offs, cols = unit_geom(u)
            pts = []
            for j in range(2):
                pt = pt_pool.tile(
                    [128, cols], bf16, tag=f"ph{u}_{j}", name=f"pt{g}_{u}_{j}", bufs=1
                )
                pts.append(pt)
            chunks = []  # (t, s_from, win, win_off, wlen)
            pos = 0
            for t in ts:
                s_from = max(t * 128, s_lo)
                rem = s_hi - s_from
                while rem:
                    wlen = min(512 - pos % 512, rem)
                    chunks.append((t, s_from, pos // 1024, pos % 1024, wlen))
                    pos += wlen
                    s_from += wlen
                    rem -= wlen
            nwin = (pos + 1023) // 1024
            covered = 0
            masks_todo = [t for t in ts if t * 128 >= s_lo]
            for w in range(nwin):
                wchunks = [c for c in chunks if c[2] == w]
                wcols = sum(c[4] for c in wchunks)
                gbase = 1024 * w
                stt = []
                for j in range(2):
                    st_t = ps_st.tile(
                        [128, 1024], f32, tag="st", name=f"st{g}_{u}_{w}_{j}"
                    )
                    stt.append(st_t)
                for t, s_from, _, woff, wlen in wchunks:
                    for j in range(2):
                        ro = j * HD
                        nc.tensor.matmul(
                            stt[j][:, woff : woff + wlen],
                            kT[ro : ro + HD, g, t * 128 : t * 128 + 128],
                            qT[ro : ro + HD, g, s_from : s_from + wlen],
                            start=True,
                            stop=True,
                        )
                for j in range(2):
                    nc.scalar.activation(
                        pts[j][:, gbase : gbase + wcols],
                        stt[j][:, 0:wcols],
                        Exp,
                        scale=1.0 / np.sqrt(HD),
                    )
                covered += wcols
                while masks_todo and offs[masks_todo[0]] + 128 <= covered:
                    t = masks_todo.pop(0)
                    for j in range(2):
                        nc.vector.tensor_tensor(
                            pts[j][:, offs[t] : offs[t] + 128],
                            pts[j][:, offs[t] : offs[t] + 128],
                            mask_sb[:],
                            mult,
                        )
                if filler is not None and (cadence == 1 or w % cadence == 1):
                    blk = filler()
                    if blk is not None:
                        blk()
            assert not masks_todo
            return pts

        def outproj_sc(dc, sc):
            """Output projection for one (128-feature, 512-token) tile + DMA."""
            ot = out_pool.tile([128, 512], f32, tag="ot", name=f"ot{dc}_{sc}")
            psum = ps_mm.tile([128, 512], f32, tag="mm", name=f"po{dc}_{sc}")
            kchain(
                psum[:],
                lambda c: wo_sb[:, c, dc * 128 : (dc + 1) * 128],
                lambda c: oT[:, c, sc * 512 : (sc + 1) * 512],
                EC,
            )
            nc.vector.tensor_scalar_add(ot[:], psum[:], bo_sb[:, dc : dc + 1])
            nc.sync.dma_start(
                out[dc * 128 : (dc + 1) * 128, sc * 512 : (sc + 1) * 512], ot[:]
            )

        def av_group(g, u, pts, j, g4, norm_q):
            """O^T accumulation for head (g, j), s-group g4. The normalize
            multiply is deferred via norm_q so the in-order DVE never blocks
            on the GpSimd broadcast while later PSUM evacuations queue."""
            s_lo, s_hi, ts, offs, cols = unit_geom(u)
            h = 2 * g + j
            ro = j * HD
            av = ps_mm.tile([128, 512], f32, tag="mm", name=f"av{g}_{u}_{j}_{g4}")
            tlist = [t for t in ts if t * 128 < (g4 + 1) * 512]
            for ci, t in enumerate(tlist):
                lo = max(g4 * 512, t * 128)
                n = (g4 + 1) * 512 - lo
                col = offs[t] + lo - max(t * 128, s_lo)
                nc.tensor.matmul(
                    av[0 : HD + 1, lo - g4 * 512 : lo - g4 * 512 + n],
                    v_sb[:, t, h, :],
                    pts[j][:, col : col + n],
                    start=(ci == 0),
                    stop=(ci == len(tlist) - 1),
                )
            # evacuate PSUM quickly (frees the accumulator slot). The denom
            # reciprocal runs on a [128,4] transpose (a [1,512] DVE op is
            # ~20x slower on hardware), then is broadcast across partitions.
            osb = ou_pool.tile([HD + 1, 512], f32, tag="ou", name=f"ou_{g}_{u}_{j}_{g4}")
            nc.vector.tensor_copy(osb[:], av[0 : HD + 1, :])
            rs = r_pool.tile([128, 4], f32, tag="rs", name=f"rs_{g}_{u}_{j}_{g4}")
            nc.gpsimd.dma_start(rs[:], osb[HD : HD + 1, :])
            rr = r_pool.tile([128, 4], f32, tag="rrt", name=f"rrt_{g}_{u}_{j}_{g4}")
            nc.vector.reciprocal(rr[:], rs[:])
            rrow = r_pool.tile([1, 512], f32, tag="rr", name=f"rr_{g}_{u}_{j}_{g4}")
            nc.gpsimd.dma_start(rrow[:], rr[:])
            r64 = r_pool.tile(
                [HD, 512], f32, tag="r64", name=f"r64_{g}_{u}_{j}_{g4}"
            )
            nc.gpsimd.partition_broadcast(r64[:], rrow[:])

            def norm():
                nc.vector.tensor_tensor(
                    oT[ro : ro + HD, g, g4 * 512 : (g4 + 1) * 512],
                    osb[0:HD, :],
                    r64[:],
                    mult,
                )
            norm_q.append(norm)

        def av_unit(g, u, pts, g4_desc=False):
            s_lo, s_hi = u * UW, min((u + 1) * UW, S)
            norm_q = []
            g4s = range(s_lo // 512, s_hi // 512)
            for g4 in (reversed(g4s) if g4_desc else g4s):
                for j in range(2):
                    av_group(g, u, pts, j, g4, norm_q)
                    while len(norm_q) > 1:
                        norm_q.pop(0)()
            while norm_q:
                norm_q.pop(0)()

        # ---- emission ----
        # Upfront: pairs 0-1 q/k for the left s-half only, with x DMAs
        # interleaved ahead of the chains that consume them. The exp table
        # preloads on ScalarE during the initial DMA wait.
        xts = {}
        xts["q0"] = proj_qk_dma("q", xq, 0, 0)
        nc.sync.dma_start(
            wkg0[:], wk[:, 0:256].rearrange("(kc p) m -> p kc m", p=128)
        )
        xts["k0"] = proj_qk_dma("k", xk, 0, 0)
        nc.sync.dma_start(bq_sb[:], bqd[:])
        nc.sync.dma_start(bk_sb[:], bkd[:])
        warm = r_pool.tile([128, 4], f32, tag="rs", name="warm")
        nc.vector.memset(warm[:], 1.0)
        nc.scalar.activation(warm[:], warm[:], Exp, scale=1.0)
        proj_qk_mms(xts["q0"], wqg0, bq_sb, qT, 0, 0, "q")
        xts["q1"] = proj_qk_dma("q", xq, 0, 1)
        proj_qk_mms(xts["k0"], wkg0, bk_sb, kT, 0, 0, "k")
        xts["k1"] = proj_qk_dma("k", xk, 0, 1)
        proj_qk_mms(xts["q1"], wqg0, bq_sb, qT, 0, 1, "q")
        proj_qk_mms(xts["k1"], wkg0, bk_sb, kT, 0, 1, "k")

        # Remaining const DMAs (needed from the first windows / v blocks on).
        nc.sync.dma_start(mask_sb[:], maskd[:])
        nc.sync.dma_start(wv_sb[:], wv.rearrange("(kc p) m -> p kc m", p=128))
        nc.sync.dma_start(bv_sb[:], bvd[:])
        nc.sync.dma_start(wo_sb[:], wo.rearrange("(ec p) d -> p ec d", p=128))
        nc.sync.dma_start(bo_sb[:], bod[:])
        nc.vector.memset(v_sb[:, :, :, HD : HD + 1], 1.0)
        wqg1, wkg1 = make_gh1_weights()

        # Filler blocks: (label, dma_fn|None, mm_fn). The x-slice DMA of the
        # next dma-bearing block is issued one block ahead of its compute so
        # the transfer hides under the previous block's matmuls.
        emitted = {"q0s0", "q0s1", "k0s0", "k0s1"}
        blocks = []
        _slices = {}

        def add_qk(gh, nm, sc):
            lbl = f"{nm}{gh}s{sc}"
            xsrc = xq if nm == "q" else xk
            wsb = (wqg0 if nm == "q" else wkg0) if gh == 0 else (
                wqg1 if nm == "q" else wkg1)
            bsb = bq_sb if nm == "q" else bk_sb
            dst = qT if nm == "q" else kT

            def dma(lbl=lbl):
                _slices[lbl] = proj_qk_dma(nm, xsrc, gh, sc)

            def mms(lbl=lbl):
                proj_qk_mms(_slices.pop(lbl), wsb, bsb, dst, gh, sc, nm)
            blocks.append((lbl, dma, mms))

        def add_v(sc, half):
            lbl = f"v{sc}"

            def dma(sc=sc):
                _slices[f"xv{sc}"] = load_xslice(xv, sc, f"xv_{sc}")

            def mms(sc=sc, half=half):
                xt = _slices[f"xv{sc}"] if half == 0 else _slices.pop(f"xv{sc}")
                proj_v_mms(xt, sc, half)
            blocks.append((lbl, dma if half == 0 else None, mms))

        # first-need order for g-major units with st-ahead: st(0,1) is
        # emitted during iteration 0, so gh0 s23 comes first.
        for nm in ("q", "k"):
            for sc in (2, 3):
                add_qk(0, nm, sc)
        for sc in (0, 1, 2, 3):
            for half in (0, 1):
                add_v(sc, half)
        for sc in (0, 1, 2, 3):
            for nm in ("q", "k"):
                add_qk(1, nm, sc)

        state = {"dma": 0, "mm": 0}

        def _advance_dma():
            # issue x DMAs up to one dma-bearing block past the next compute
            want = state["mm"] + 2
            while state["dma"] < min(want, len(blocks)):
                i = state["dma"]
                if blocks[i][1] is not None:
                    blocks[i][1]()
                state["dma"] += 1

        def run_next_block():
            i = state["mm"]
            lbl, _, mms = blocks[i]
            state["mm"] += 1
            _advance_dma()
            mms()
            emitted.add(lbl)

        _advance_dma()

        def take_filler():
            if state["mm"] < len(blocks):
                return run_next_block
            if fill_q:
                return fill_q.popleft()[1]
            return None

        fill_q = deque()

        def push(label, fn):
            def run(label=label, fn=fn):
                fn()
                emitted.add(label)
            fill_q.append((label, run))

        def prep(labels):
            while state["mm"] < len(blocks) and not all(
                l in emitted for l in labels
            ):
                run_next_block()
            while fill_q and not all(l in emitted for l in labels):
                fill_q.popleft()[1]()

        def st_needs(g, u):
            gh = g // 2
            scs = (0, 1) if u == 0 else (0, 1, 2, 3)
            qscs = (0, 1) if u == 0 else (2, 3)
            return [f"q{gh}s{sc}" for sc in qscs] + [f"k{gh}s{sc}" for sc in scs]

        def av_needs(u):
            scs = (0, 1) if u == 0 else (2, 3)
            return [f"v{sc}" for sc in scs]

        units = [(g, u) for g in range(G) for u in range(NU)]

        reserve = []

        def push_outproj_left():
            for sc in range(UW // 512):
                for dc in range(D // 128):
                    if sc == 1 and dc >= 4:
                        # held back: emitted between the last unit's windows
                        # and its AV groups, covering the ScalarE exp backlog
                        reserve.append(lambda dc=dc, sc=sc: outproj_sc(dc, sc))
                    else:
                        push(f"op{dc}_{sc}", lambda dc=dc, sc=sc: outproj_sc(dc, sc))

        # Software pipeline: unit i+1's score windows are emitted before unit
        # i's AV groups so ScalarE exps overlap the AV matmuls. At (3,0) the
        # order flips so the left-half output projection unlocks before the
        # final unit's windows need fillers.
        pts_next = st_unit(*units[0], take_filler, cadence=2)
        for i, (g, u) in enumerate(units):
            pts_cur = pts_next
            last = i == len(units) - 1
            ahead = not last and units[i + 1]
            if ahead and (g, u) != (G - 1, 0):
                prep(st_needs(*ahead))
                pts_next = st_unit(*ahead, take_filler,
                                   cadence=2 if i + 1 < 4 else 1)
            prep(av_needs(u))
            if last:
                # reserved fillers cover the ScalarE exp backlog of the last
                # windows; AV groups run g4-descending so outproj(sc3) hides
                # under the g4=2 AV+normalize chain and outproj(sc2) under
                # sc3's — the tail never waits a full r-dance latency.
                while fill_q:
                    fill_q.popleft()[1]()
                for fn in reserve:
                    fn()
                reserve.clear()
                av_unit(g, u, pts_cur, g4_desc=True)
                for dc in range(D // 128):
                    outproj_sc(dc, 3)
                for dc in range(D // 128):
                    outproj_sc(dc, 2)
            else:
                av_unit(g, u, pts_cur)
            if (g, u) == (G - 1, 0):
                push_outproj_left()
                prep(st_needs(*ahead))
                pts_next = st_unit(*ahead, take_filler)
        while fill_q:
            fill_q.popleft()[1]()

    nc.compile()
    return nc


def core_inputs(queries, keys, values, Wq, bq, Wk, bk, Wv, bv, Wo, bo, b, hh):
    """Build the per-core input map (host-side sharding + bf16 cast)."""
    D = queries.shape[2]
    hs = slice(hh * NH, hh * NH + NH)

    def xt(x):
        return np.ascontiguousarray(x[b].astype(BF16).T)

    def wcat(W):
        return np.ascontiguousarray(
            np.transpose(W[hs], (1, 0, 2)).reshape(D, NH * HD).astype(BF16)
        )

    def bstack(bias):
        return np.ascontiguousarray(
            bias[hs].reshape(G, 128).T.astype(np.float32)
        )

    mask = np.triu(np.ones((128, 128), np.float32)).astype(BF16)
    return {
        "xq": xt(queries),
        "xk": xt(keys),
        "xv": xt(values),
        "wq": wcat(Wq),
        "wk": wcat(Wk),
        "wv": wcat(Wv),
        "wo": np.ascontiguousarray(Wo[hh * NH * HD : (hh + 1) * NH * HD].astype(BF16)),
        "bq": bstack(bq),
        "bk": bstack(bk),
        "bv": np.ascontiguousarray(
            np.broadcast_to(bv[hs].reshape(1, NH, HD), (128, NH, HD)).astype(np.float32)
        ),
        "bo": np.ascontiguousarray(
            (bo.reshape(D // 128, 128) / 2.0).T.astype(np.float32)
        ),
        "mask": mask,
    }


_NC_CACHE = {}


def _get_nc(S, D):
    key = (S, D)
    if key not in _NC_CACHE:
        _NC_CACHE[key] = build_nc(S, D)
    return _NC_CACHE[key]


def kernel(keys, queries, values, Wq, bq, Wk, bk, Wv, bv, Wo, bo, _trace=False):
    keys, queries, values = (np.asarray(a) for a in (keys, queries, values))
    Wq, bq, Wk, bk, Wv, bv, Wo, bo = (
        np.asarray(a) for a in (Wq, bq, Wk, bk, Wv, bv, Wo, bo)
    )
    B, S, D = queries.shape
    nc = _get_nc(S, D)

    in_maps = [
        core_inputs(queries, keys, values, Wq, bq, Wk, bk, Wv, bv, Wo, bo, c // 2, c % 2)
        for c in range(8)
    ]
    from concourse.bass_utils import run_bass_kernel_spmd

    res = run_bass_kernel_spmd(
        nc, in_maps, core_ids=list(range(8)), trace=_trace
    )
    kernel.last_result = res
    outs = [r["out"] for r in res.results]
    out = np.empty((B, S, D), np.float32)
    for b in range(B):
        out[b] = (outs[2 * b] + outs[2 * b + 1]).T
    return out
